# revision 12
# baseline (speedup 1.0000x reference)
"""DKVMN forward kernel for 8 Trainium2 NeuronCores.

Data-parallel over batch: B=128 -> 16 per core. Per-core state
v[d=128 partitions, (b,m)=16*50=800 free] f32 in SBUF.

Per scan step t:
  w_exp = ones^T @ attn_row_t        (PE, f32r exact broadcast, PSUM)
  z     = v * w_exp                  (DVE)
  readT = reduce_m(z)                (DVE)          [128,16]
  hT    = tanh(W1r^T readT + W1q^T qeT + b1)   (PE + ACT)
  eT    = sigmoid(W2er^T hT + eb)    (PE + ACT)    W2er = vu_w2 @ er_w (host)
  aT    = tanh(W2ad^T hT + ab)       (PE + ACT)
  t1    = z * bcast(eT)              (DVE)
  t2    = w_exp * bcast(aT)          (DVE, -> SBUF)
  g     = t1 - v                     (GPSIMD)
  v     = t2 - g                     (GPSIMD)      = v - z*e + w*a
"""

import os
import numpy as np
from contextlib import ExitStack

import concourse.bass as bass
import concourse.bacc as bacc
import concourse.mybir as mybir
import concourse.tile as tile
import concourse.bass_utils as bass_utils
from concourse.masks import make_identity

B, S, M, D, NQ = 128, 100, 50, 128, 10000
NCORES = 8
BC = B // NCORES          # 16 batch rows per core
BM = BC * M               # 800
NQTILES = (S * BC + 127) // 128   # 13 gather tiles
QCOLS = NQTILES * 128     # 1664

F32 = mybir.dt.float32
F32R = mybir.dt.float32r
I32 = mybir.dt.int32
AF = mybir.ActivationFunctionType
OP = mybir.AluOpType
AX = mybir.AxisListType

_CACHE = {}


def _build_program():
    if "nc" in _CACHE:
        return _CACHE["nc"]

    nc = bacc.Bacc("TRN2", target_bir_lowering=False, debug=False,
                   enable_asserts=False, num_devices=NCORES)

    dram_in = {}
    for name, shape, dt in [
        ("emb", [NQ, D], F32),
        ("qidx", [128, NQTILES], I32),
        ("kT", [D, M], F32),
        ("w1r", [D, D], F32), ("w1q", [D, D], F32),
        ("w2er", [D, D], F32), ("w2ad", [D, D], F32),
        ("b1", [D, 1], F32), ("eb", [D, 1], F32), ("ab", [D, 1], F32),
        ("ow1r", [D, D], F32), ("ow1q", [D, D], F32),
        ("ob1", [D, 1], F32), ("ow2", [D, 1], F32), ("ob2", [1, 1], F32),
    ]:
        dram_in[name] = nc.dram_tensor(name, shape, dt, kind="ExternalInput").ap()
    pred_out = nc.dram_tensor("pred", [1, BC], F32, kind="ExternalOutput").ap()

    with tile.TileContext(nc) as tc, ExitStack() as ctx:
        persist = ctx.enter_context(tc.tile_pool(name="persist", bufs=1))

        # ---- persistent SBUF tiles ----
        kT = persist.tile([D, M], F32, tag="kT")
        w1r = persist.tile([D, D], F32, tag="w1r")
        w1q = persist.tile([D, D], F32, tag="w1q")
        w2er = persist.tile([D, D], F32, tag="w2er")
        w2ad = persist.tile([D, D], F32, tag="w2ad")
        b1 = persist.tile([D, 1], F32, tag="b1")
        eb = persist.tile([D, 1], F32, tag="eb")
        ab = persist.tile([D, 1], F32, tag="ab")
        ow1r = persist.tile([D, D], F32, tag="ow1r")
        ow1q = persist.tile([D, D], F32, tag="ow1q")
        ob1 = persist.tile([D, 1], F32, tag="ob1")
        ow2 = persist.tile([D, 1], F32, tag="ow2")
        ob2 = persist.tile([1, 1], F32, tag="ob2")
        idx = persist.tile([128, NQTILES], I32, tag="idx")
        ident = persist.tile([128, 128], F32, tag="ident")
        identr = persist.tile([128, 128], F32R, tag="identr")
        attnr = persist.tile([S, BM], F32R, tag="attnr")
        qT = persist.tile([D, QCOLS], F32, tag="qT")
        attn = persist.tile([S, BM], F32, tag="attn")
        v = persist.tile([D, BM], F32, tag="v")

        for nm, t in [("kT", kT), ("w1r", w1r), ("w1q", w1q), ("w2er", w2er),
                      ("w2ad", w2ad), ("b1", b1), ("eb", eb), ("ab", ab),
                      ("ow1r", ow1r), ("ow1q", ow1q), ("ob1", ob1),
                      ("ow2", ow2), ("ob2", ob2), ("qidx", idx)]:
            nc.sync.dma_start(t[:], dram_in[nm][:])
        make_identity(nc, ident[:])
        nc.vector.tensor_copy(identr[:], ident[:])
        nc.vector.memset(v[:], 0.0)

        # ---- phase 1: gather q_emb rows and transpose into qT ----
        with tc.tile_pool(name="gather", bufs=3) as gpool, \
             tc.tile_pool(name="tpsum", bufs=4, space="PSUM") as tpsum:
            for j in range(NQTILES):
                qg = gpool.tile([128, D], F32, tag="qg")
                nc.gpsimd.indirect_dma_start(
                    out=qg[:], out_offset=None,
                    in_=dram_in["emb"][:],
                    in_offset=bass.IndirectOffsetOnAxis(ap=idx[:, j:j + 1], axis=0),
                )
                tp = tpsum.tile([128, 128], F32, tag="tp")
                nc.tensor.transpose(tp[:], qg[:], ident[:])
                if j % 2 == 0:
                    nc.vector.tensor_copy(qT[:, j * 128:(j + 1) * 128], tp[:])
                else:
                    nc.scalar.copy(qT[:, j * 128:(j + 1) * 128], tp[:])

        # ---- phase 2: scores + softmax -> attn[s, (b,m)] ----
        with tc.tile_pool(name="spsum", bufs=4, space="PSUM") as spsum:
            for b in range(BC):
                sc = spsum.tile([S, M], F32, tag="sc")
                qTb = qT[:, b:S * BC:BC]          # [128, 100] strided (s,b) layout
                nc.tensor.matmul(sc[:], qTb, kT[:], start=True, stop=True)
                if b % 2 == 0:
                    nc.vector.tensor_copy(attn[:, b * M:(b + 1) * M], sc[:])
                else:
                    nc.scalar.copy(attn[:, b * M:(b + 1) * M], sc[:])

        with tc.tile_pool(name="smx", bufs=1) as smx:
            a3 = attn[:].rearrange("p (b m) -> p b m", b=BC)
            mx = smx.tile([S, BC], F32, tag="mx")
            nc.vector.tensor_reduce(mx[:], a3, axis=AX.X, op=OP.max)
            mxb = mx[:, :, None].broadcast_to([S, BC, M])
            nc.vector.tensor_tensor(a3, a3, mxb, op=OP.subtract)
            nc.scalar.activation(attn[:], attn[:], AF.Exp)
            sm = smx.tile([S, BC], F32, tag="sm")
            nc.vector.tensor_reduce(sm[:], a3, axis=AX.X, op=OP.add)
            rec = smx.tile([S, BC], F32, tag="rec")
            nc.vector.reciprocal(rec[:], sm[:])
            recb = rec[:, :, None].broadcast_to([S, BC, M])
            nc.vector.tensor_tensor(a3, a3, recb, op=OP.mult)
            nc.vector.tensor_copy(attnr[:], attn[:])

        # ---- phase 3: the scan (two independent batch groups, interleaved) ----
        # Group g owns batch rows [8g, 8g+8) -> v columns [400g, 400g+400).
        GW = 400          # group width in v-columns
        GB2 = BC // 2     # 8 batch rows per group
        with tc.tile_pool(name="wide", bufs=3) as wide, \
             tc.tile_pool(name="small", bufs=4) as small, \
             tc.tile_pool(name="wexp", bufs=4, space="PSUM") as wexpp, \
             tc.tile_pool(name="mlp", bufs=2, space="PSUM") as mlpp, \
             tc.tile_pool(name="mlp2", bufs=1, space="PSUM") as mlpp2:

            def g_read(t, g):
                """wexp select-bcast + z + readT for group g at step t."""
                c0 = g * GW
                sel = identr[0:S, t:t + 1].broadcast_to([S, D])
                w = wexpp.tile([D, GW], F32, tag="w")
                nc.tensor.matmul(w[:], sel, attnr[:, c0:c0 + GW],
                                 start=True, stop=True)
                z = wide.tile([D, GW], F32, tag=f"z{g}")
                nc.vector.tensor_tensor(z[:], v[:, c0:c0 + GW], w[:], op=OP.mult)
                readT = small.tile([D, GB2], F32, tag=f"r{g}")
                z3 = z[:].rearrange("p (b m) -> p b m", b=GB2)
                nc.vector.tensor_reduce(readT[:], z3, axis=AX.X, op=OP.add)
                return z, w, readT

            def g_gates(t, g, readT):
                """MLP + gates for group g; returns eT, aT ([D, 8])."""
                qeT = qT[:, t * BC + g * GB2: t * BC + (g + 1) * GB2]
                hps = mlpp.tile([D, GB2], F32, tag="hps")
                nc.tensor.matmul(hps[:], w1r[:], readT[:], start=True, stop=False)
                nc.tensor.matmul(hps[:], w1q[:], qeT, start=False, stop=True)
                hT = small.tile([D, GB2], F32, tag=f"h{g}")
                nc.scalar.activation(hT[:], hps[:], AF.Tanh, bias=b1[:])
                eps = mlpp2.tile([D, GB2], F32, tag="eps")
                nc.tensor.matmul(eps[:], w2er[:], hT[:], start=True, stop=True)
                eT = small.tile([D, GB2], F32, tag=f"e{g}")
                nc.scalar.activation(eT[:], eps[:], AF.Sigmoid, bias=eb[:])
                aps = mlpp2.tile([D, GB2], F32, tag="aps")
                nc.tensor.matmul(aps[:], w2ad[:], hT[:], start=True, stop=True)
                aT = small.tile([D, GB2], F32, tag=f"a{g}")
                nc.scalar.activation(aT[:], aps[:], AF.Tanh, bias=ab[:])
                return eT, aT

            def g_update(g, z, w, eT, aT):
                """v[g] <- v[g] - z*E + w*A  via d = t1 - t2; v -= d."""
                c0 = g * GW
                z3 = z[:].rearrange("p (b m) -> p b m", b=GB2)
                ebc = eT[:, :, None].broadcast_to([D, GB2, M])
                t1 = wide.tile([D, GW], F32, tag=f"t1{g}")
                nc.gpsimd.tensor_tensor(t1[:].rearrange("p (b m) -> p b m", b=GB2),
                                        z3, ebc, op=OP.mult)
                abc = aT[:, :, None].broadcast_to([D, GB2, M])
                t2 = wide.tile([D, GW], F32, tag=f"t2{g}")
                nc.vector.tensor_tensor(
                    t2[:].rearrange("p (b m) -> p b m", b=GB2),
                    w[:].rearrange("p (b m) -> p b m", b=GB2), abc, op=OP.mult)
                d = wide.tile([D, GW], F32, tag=f"d{g}")
                nc.gpsimd.tensor_tensor(d[:], t1[:], t2[:], op=OP.subtract)
                nc.vector.tensor_tensor(v[:, c0:c0 + GW], v[:, c0:c0 + GW],
                                        d[:], op=OP.subtract)

            state = [None, None]
            for t in range(S):
                for g in (0, 1):
                    z, w, readT = g_read(t, g)
                    eT, aT = g_gates(t, g, readT)
                    state[g] = (z, w, eT, aT)
                for g in (0, 1):
                    g_update(g, *state[g])

            # ---- final prediction ----
            readF = small.tile([D, BC], F32, tag="readF")
            for g in (0, 1):
                c0 = g * GW
                sel = identr[0:S, S - 1:S].broadcast_to([S, D])
                w = wexpp.tile([D, GW], F32, tag="w")
                nc.tensor.matmul(w[:], sel, attnr[:, c0:c0 + GW],
                                 start=True, stop=True)
                zf = wide.tile([D, GW], F32, tag=f"z{g}")
                nc.vector.tensor_tensor(zf[:], v[:, c0:c0 + GW], w[:], op=OP.mult)
                z3 = zf[:].rearrange("p (b m) -> p b m", b=GB2)
                nc.vector.tensor_reduce(readF[:, g * GB2:(g + 1) * GB2], z3,
                                        axis=AX.X, op=OP.add)
            qeT = qT[:, (S - 1) * BC:S * BC]
            h2ps = mlpp.tile([D, BC], F32, tag="hps")
            nc.tensor.matmul(h2ps[:], ow1r[:], readF[:], start=True, stop=False)
            nc.tensor.matmul(h2ps[:], ow1q[:], qeT, start=False, stop=True)
            h2 = small.tile([D, BC], F32, tag="h0")
            nc.scalar.activation(h2[:], h2ps[:], AF.Relu, bias=ob1[:])
            pps = mlpp2.tile([1, BC], F32, tag="eps")
            nc.tensor.matmul(pps[:], ow2[:], h2[:], start=True, stop=True)
            ps = small.tile([1, BC], F32, tag="pred")
            nc.scalar.activation(ps[:], pps[:], AF.Sigmoid, bias=ob2[:])
            nc.sync.dma_start(pred_out[:], ps[:])

    nc.compile()
    _CACHE["nc"] = nc
    return nc


def _host_inputs(inputs):
    """Per-core input maps from the full problem inputs."""
    q = np.asarray(inputs["question_seq"]).astype(np.int64)
    emb = np.ascontiguousarray(np.asarray(inputs["emb"], dtype=np.float32))
    key_matrix = np.asarray(inputs["key_matrix"], dtype=np.float32)
    vu_w1 = np.asarray(inputs["vu_w1"], dtype=np.float32)
    vu_b1 = np.asarray(inputs["vu_b1"], dtype=np.float32)
    vu_w2 = np.asarray(inputs["vu_w2"], dtype=np.float32)
    vu_b2 = np.asarray(inputs["vu_b2"], dtype=np.float32)
    er_w = np.asarray(inputs["er_w"], dtype=np.float32)
    er_b = np.asarray(inputs["er_b"], dtype=np.float32)
    ad_w = np.asarray(inputs["ad_w"], dtype=np.float32)
    ad_b = np.asarray(inputs["ad_b"], dtype=np.float32)
    out_w1 = np.asarray(inputs["out_w1"], dtype=np.float32)
    out_b1 = np.asarray(inputs["out_b1"], dtype=np.float32)
    out_w2 = np.asarray(inputs["out_w2"], dtype=np.float32)
    out_b2 = np.asarray(inputs["out_b2"], dtype=np.float32)

    w2er = (vu_w2.astype(np.float64) @ er_w.astype(np.float64)).astype(np.float32)
    w2ad = (vu_w2.astype(np.float64) @ ad_w.astype(np.float64)).astype(np.float32)
    ebf = (vu_b2.astype(np.float64) @ er_w.astype(np.float64) + er_b).astype(np.float32)
    abf = (vu_b2.astype(np.float64) @ ad_w.astype(np.float64) + ad_b).astype(np.float32)

    shared = {
        "emb": emb,
        "kT": np.ascontiguousarray(key_matrix.T),
        "w1r": np.ascontiguousarray(vu_w1[:D]),
        "w1q": np.ascontiguousarray(vu_w1[D:]),
        "w2er": w2er, "w2ad": w2ad,
        "b1": vu_b1.reshape(D, 1), "eb": ebf.reshape(D, 1), "ab": abf.reshape(D, 1),
        "ow1r": np.ascontiguousarray(out_w1[:D]),
        "ow1q": np.ascontiguousarray(out_w1[D:]),
        "ob1": out_b1.reshape(D, 1),
        "ow2": np.ascontiguousarray(out_w2.reshape(D, 1)),
        "ob2": out_b2.reshape(1, 1),
    }
    in_maps = []
    for c in range(NCORES):
        qidx = np.zeros((128, NQTILES), np.int32)
        for j in range(NQTILES):
            for p in range(128):
                n = j * 128 + p
                if n < S * BC:
                    s, bl = divmod(n, BC)
                    qidx[p, j] = q[c * BC + bl, s]
        m = dict(shared)
        m["qidx"] = qidx
        in_maps.append(m)
    return in_maps


def _install_ntff_shim():
    # Optional: enables NTFF hardware profiling under axon when tracing is
    # requested. Harmless no-op if the pieces are missing.
    import types, sys
    if "antenv.axon_hooks" in sys.modules:
        return
    try:
        import antenv
        from trn_agent_boot.trn_boot import _ntff_profile_via_ctypes
        hook = _ntff_profile_via_ctypes("/opt/axon/libaxon_pjrt.so")
        mod = types.ModuleType("antenv.axon_hooks")
        state = {"hook": hook}
        mod.get_axon_ntff_profile_hook = lambda: state["hook"]
        mod.set_axon_ntff_profile_hook = lambda h: state.update(hook=h)
        sys.modules["antenv.axon_hooks"] = mod
        antenv.axon_hooks = mod
    except Exception:
        pass


def kernel(**inputs) -> np.ndarray:
    if bool(int(os.environ.get("DKVMN_TRACE", "0"))):
        _install_ntff_shim()
    nc = _build_program()
    in_maps = _host_inputs(inputs)
    res = bass_utils.run_bass_kernel_spmd(
        nc, in_maps, core_ids=list(range(NCORES)),
        trace=bool(int(os.environ.get("DKVMN_TRACE", "0"))),
    )
    _CACHE["last_results"] = res
    pred = np.concatenate([res.results[c]["pred"].reshape(BC) for c in range(NCORES)])
    return pred.astype(np.float32)


# revision 16
# speedup vs baseline: 1.2530x; 1.2530x over previous
"""DKVMN forward kernel for 8 Trainium2 NeuronCores.

Data-parallel over batch: B=128 -> 16 per core. Per-core state
v[d=128 partitions, (b,m)=16*50=800 free] f32 in SBUF.

Per scan step t:
  w_exp = ones^T @ attn_row_t        (PE, f32r exact broadcast, PSUM)
  z     = v * w_exp                  (DVE)
  readT = reduce_m(z)                (DVE)          [128,16]
  hT    = tanh(W1r^T readT + W1q^T qeT + b1)   (PE + ACT)
  eT    = sigmoid(W2er^T hT + eb)    (PE + ACT)    W2er = vu_w2 @ er_w (host)
  aT    = tanh(W2ad^T hT + ab)       (PE + ACT)
  t1    = z * bcast(eT)              (DVE)
  t2    = w_exp * bcast(aT)          (DVE, -> SBUF)
  g     = t1 - v                     (GPSIMD)
  v     = t2 - g                     (GPSIMD)      = v - z*e + w*a
"""

import os
import numpy as np
import ml_dtypes
from contextlib import ExitStack

import concourse.bass as bass
import concourse.bacc as bacc
import concourse.mybir as mybir
import concourse.tile as tile
import concourse.bass_utils as bass_utils
from concourse.masks import make_identity

B, S, M, D, NQ = 128, 100, 50, 128, 10000
NCORES = 8
BC = B // NCORES          # 16 batch rows per core
BM = BC * M               # 800
NQTILES = (S * BC + 127) // 128   # 13 gather tiles
QCOLS = NQTILES * 128     # 1664

F32 = mybir.dt.float32
F32R = mybir.dt.float32r
BF16 = mybir.dt.bfloat16
I32 = mybir.dt.int32
AF = mybir.ActivationFunctionType
OP = mybir.AluOpType
AX = mybir.AxisListType

_CACHE = {}


def _build_program():
    if "nc" in _CACHE:
        return _CACHE["nc"]

    nc = bacc.Bacc("TRN2", target_bir_lowering=False, debug=False,
                   enable_asserts=False, num_devices=NCORES)

    dram_in = {}
    for name, shape, dt in [
        ("emb", [NQ, D], F32),
        ("qidx", [128, NQTILES], I32),
        ("kT", [D, M], F32),
        ("w1r", [D, D], BF16), ("w1q", [D, D], BF16),
        ("w2er", [D, D], BF16), ("w2ad", [D, D], BF16),
        ("b1", [D, 1], F32), ("eb", [D, 1], F32), ("ab", [D, 1], F32),
        ("ow1r", [D, D], F32), ("ow1q", [D, D], F32),
        ("ob1", [D, 1], F32), ("ow2", [D, 1], F32), ("ob2", [1, 1], F32),
    ]:
        dram_in[name] = nc.dram_tensor(name, shape, dt, kind="ExternalInput").ap()
    pred_out = nc.dram_tensor("pred", [1, BC], F32, kind="ExternalOutput").ap()

    with tile.TileContext(nc) as tc, ExitStack() as ctx:
        persist = ctx.enter_context(tc.tile_pool(name="persist", bufs=1))

        # ---- persistent SBUF tiles ----
        kT = persist.tile([D, M], F32, tag="kT")
        w1r = persist.tile([D, D], BF16, tag="w1r")
        w1q = persist.tile([D, D], BF16, tag="w1q")
        w2er = persist.tile([D, D], BF16, tag="w2er")
        w2ad = persist.tile([D, D], BF16, tag="w2ad")
        b1 = persist.tile([D, 1], F32, tag="b1")
        eb = persist.tile([D, 1], F32, tag="eb")
        ab = persist.tile([D, 1], F32, tag="ab")
        ow1r = persist.tile([D, D], F32, tag="ow1r")
        ow1q = persist.tile([D, D], F32, tag="ow1q")
        ob1 = persist.tile([D, 1], F32, tag="ob1")
        ow2 = persist.tile([D, 1], F32, tag="ow2")
        ob2 = persist.tile([1, 1], F32, tag="ob2")
        idx = persist.tile([128, NQTILES], I32, tag="idx")
        ident = persist.tile([128, 128], F32, tag="ident")
        identr = persist.tile([128, 128], F32R, tag="identr")
        attnr = persist.tile([S, BM], F32R, tag="attnr")
        qT = persist.tile([D, QCOLS], F32, tag="qT")
        qTb = persist.tile([D, QCOLS], BF16, tag="qTb")
        attn = persist.tile([S, BM], F32, tag="attn")
        vpp = [[persist.tile([D, 400], F32, name=f"v{g}p{p}", tag=f"v{g}p{p}")
                for p in (0, 1)] for g in (0, 1)]

        for nm, t in [("kT", kT), ("w1r", w1r), ("w1q", w1q), ("w2er", w2er),
                      ("w2ad", w2ad), ("b1", b1), ("eb", eb), ("ab", ab),
                      ("ow1r", ow1r), ("ow1q", ow1q), ("ob1", ob1),
                      ("ow2", ow2), ("ob2", ob2), ("qidx", idx)]:
            nc.sync.dma_start(t[:], dram_in[nm][:])
        make_identity(nc, ident[:])
        nc.vector.tensor_copy(identr[:], ident[:])
        nc.vector.memset(vpp[0][0][:], 0.0)
        nc.vector.memset(vpp[1][0][:], 0.0)

        # ---- phase 1: gather q_emb rows and transpose into qT ----
        with tc.tile_pool(name="gather", bufs=3) as gpool, \
             tc.tile_pool(name="tpsum", bufs=4, space="PSUM") as tpsum:
            for j in range(NQTILES):
                qg = gpool.tile([128, D], F32, tag="qg")
                nc.gpsimd.indirect_dma_start(
                    out=qg[:], out_offset=None,
                    in_=dram_in["emb"][:],
                    in_offset=bass.IndirectOffsetOnAxis(ap=idx[:, j:j + 1], axis=0),
                )
                tp = tpsum.tile([128, 128], F32, tag="tp")
                nc.tensor.transpose(tp[:], qg[:], ident[:])
                if j % 2 == 0:
                    nc.vector.tensor_copy(qT[:, j * 128:(j + 1) * 128], tp[:])
                else:
                    nc.scalar.copy(qT[:, j * 128:(j + 1) * 128], tp[:])

        nc.scalar.copy(qTb[:], qT[:])

        # ---- phase 2: scores + softmax -> attn[s, (b,m)] ----
        with tc.tile_pool(name="spsum", bufs=4, space="PSUM") as spsum:
            for b in range(BC):
                sc = spsum.tile([S, M], F32, tag="sc")
                qTsl = qT[:, b:S * BC:BC]         # [128, 100] strided (s,b) layout
                nc.tensor.matmul(sc[:], qTsl, kT[:], start=True, stop=True)
                if b % 2 == 0:
                    nc.vector.tensor_copy(attn[:, b * M:(b + 1) * M], sc[:])
                else:
                    nc.scalar.copy(attn[:, b * M:(b + 1) * M], sc[:])

        with tc.tile_pool(name="smx", bufs=1) as smx:
            a3 = attn[:].rearrange("p (b m) -> p b m", b=BC)
            mx = smx.tile([S, BC], F32, tag="mx")
            nc.vector.tensor_reduce(mx[:], a3, axis=AX.X, op=OP.max)
            mxb = mx[:, :, None].broadcast_to([S, BC, M])
            nc.vector.tensor_tensor(a3, a3, mxb, op=OP.subtract)
            nc.scalar.activation(attn[:], attn[:], AF.Exp)
            sm = smx.tile([S, BC], F32, tag="sm")
            nc.vector.tensor_reduce(sm[:], a3, axis=AX.X, op=OP.add)
            rec = smx.tile([S, BC], F32, tag="rec")
            nc.vector.reciprocal(rec[:], sm[:])
            recb = rec[:, :, None].broadcast_to([S, BC, M])
            nc.vector.tensor_tensor(a3, a3, recb, op=OP.mult)
            nc.vector.tensor_copy(attnr[:], attn[:])

        # ---- phase 3: the scan (two independent batch groups, interleaved) ----
        # Group g owns batch rows [8g, 8g+8) -> v columns [400g, 400g+400).
        GW = 400          # group width in v-columns
        GB2 = BC // 2     # 8 batch rows per group
        with tc.tile_pool(name="wide", bufs=3) as wide, \
             tc.tile_pool(name="small", bufs=4) as small, \
             tc.tile_pool(name="wexp", bufs=4, space="PSUM") as wexpp, \
             tc.tile_pool(name="mlp", bufs=2, space="PSUM") as mlpp, \
             tc.tile_pool(name="mlp2", bufs=1, space="PSUM") as mlpp2:

            def g_read(t, g, vcur):
                """wexp select-bcast + z + readT for group g at step t."""
                c0 = g * GW
                sel = identr[0:S, t:t + 1].broadcast_to([S, D])
                w = wexpp.tile([D, GW], F32, tag="w")
                nc.tensor.matmul(w[:], sel, attnr[:, c0:c0 + GW],
                                 start=True, stop=True)
                z = wide.tile([D, GW], F32, tag=f"z{g}")
                nc.vector.tensor_tensor(z[:], vcur[:], w[:], op=OP.mult)
                readT = small.tile([D, GB2], F32, tag=f"r{g}")
                z3 = z[:].rearrange("p (b m) -> p b m", b=GB2)
                nc.vector.tensor_reduce(readT[:], z3, axis=AX.X, op=OP.add)
                rbf = small.tile([D, GB2], BF16, tag=f"rb{g}")
                nc.vector.tensor_copy(rbf[:], readT[:])
                return z, w, rbf

            def g_gates(t, g, readT):
                """MLP + gates for group g; returns eT, aT ([D, 8])."""
                qeT = qTb[:, t * BC + g * GB2: t * BC + (g + 1) * GB2]
                hps = mlpp.tile([D, GB2], F32, tag="hps")
                nc.tensor.matmul(hps[:], w1r[:], readT[:], start=True, stop=False)
                nc.tensor.matmul(hps[:], w1q[:], qeT, start=False, stop=True)
                hT = small.tile([D, GB2], BF16, tag=f"h{g}")
                nc.scalar.activation(hT[:], hps[:], AF.Tanh, bias=b1[:])
                eps = mlpp2.tile([D, GB2], F32, tag="eps")
                nc.tensor.matmul(eps[:], w2er[:], hT[:], start=True, stop=True)
                eT = small.tile([D, GB2], F32, tag=f"e{g}")
                nc.scalar.activation(eT[:], eps[:], AF.Sigmoid, bias=eb[:])
                aps = mlpp2.tile([D, GB2], F32, tag="aps")
                nc.tensor.matmul(aps[:], w2ad[:], hT[:], start=True, stop=True)
                aT = small.tile([D, GB2], F32, tag=f"a{g}")
                nc.scalar.activation(aT[:], aps[:], AF.Tanh, bias=ab[:])
                return eT, aT

            def g_update(g, z, w, eT, aT, vcur, vnext):
                """v[g] <- v[g] - z*E + w*A  via d = t1 - t2; v -= d."""
                c0 = g * GW
                z3 = z[:].rearrange("p (b m) -> p b m", b=GB2)
                ebc = eT[:, :, None].broadcast_to([D, GB2, M])
                t1 = wide.tile([D, GW], F32, tag=f"t1{g}")
                nc.gpsimd.tensor_tensor(t1[:].rearrange("p (b m) -> p b m", b=GB2),
                                        z3, ebc, op=OP.mult)
                abc = aT[:, :, None].broadcast_to([D, GB2, M])
                t2 = wide.tile([D, GW], F32, tag=f"t2{g}")
                nc.vector.tensor_tensor(
                    t2[:].rearrange("p (b m) -> p b m", b=GB2),
                    w[:].rearrange("p (b m) -> p b m", b=GB2), abc, op=OP.mult)
                d = wide.tile([D, GW], F32, tag=f"d{g}")
                nc.gpsimd.tensor_tensor(d[:], t1[:], t2[:], op=OP.subtract)
                nc.vector.tensor_tensor(vnext[:], vcur[:], d[:], op=OP.subtract)

            state = [None, None]
            for t in range(S):
                pc, pn = t % 2, (t + 1) % 2
                for g in (0, 1):
                    z, w, readT = g_read(t, g, vpp[g][pc])
                    eT, aT = g_gates(t, g, readT)
                    state[g] = (z, w, eT, aT)
                for g in (0, 1):
                    g_update(g, *state[g], vpp[g][pc], vpp[g][pn])

            # ---- final prediction ----
            readF = small.tile([D, BC], F32, tag="readF")
            for g in (0, 1):
                c0 = g * GW
                sel = identr[0:S, S - 1:S].broadcast_to([S, D])
                w = wexpp.tile([D, GW], F32, tag="w")
                nc.tensor.matmul(w[:], sel, attnr[:, c0:c0 + GW],
                                 start=True, stop=True)
                zf = wide.tile([D, GW], F32, tag=f"z{g}")
                nc.vector.tensor_tensor(zf[:], vpp[g][S % 2][:], w[:], op=OP.mult)
                z3 = zf[:].rearrange("p (b m) -> p b m", b=GB2)
                nc.vector.tensor_reduce(readF[:, g * GB2:(g + 1) * GB2], z3,
                                        axis=AX.X, op=OP.add)
            qeT = qT[:, (S - 1) * BC:S * BC]
            h2ps = mlpp.tile([D, BC], F32, tag="hps")
            nc.tensor.matmul(h2ps[:], ow1r[:], readF[:], start=True, stop=False)
            nc.tensor.matmul(h2ps[:], ow1q[:], qeT, start=False, stop=True)
            h2 = small.tile([D, BC], F32, tag="h0")
            nc.scalar.activation(h2[:], h2ps[:], AF.Relu, bias=ob1[:])
            pps = mlpp2.tile([1, BC], F32, tag="eps")
            nc.tensor.matmul(pps[:], ow2[:], h2[:], start=True, stop=True)
            ps = small.tile([1, BC], F32, tag="pred")
            nc.scalar.activation(ps[:], pps[:], AF.Sigmoid, bias=ob2[:])
            nc.sync.dma_start(pred_out[:], ps[:])

    nc.compile()
    _CACHE["nc"] = nc
    return nc


def _host_inputs(inputs):
    """Per-core input maps from the full problem inputs."""
    q = np.asarray(inputs["question_seq"]).astype(np.int64)
    emb = np.ascontiguousarray(np.asarray(inputs["emb"], dtype=np.float32))
    key_matrix = np.asarray(inputs["key_matrix"], dtype=np.float32)
    vu_w1 = np.asarray(inputs["vu_w1"], dtype=np.float32)
    vu_b1 = np.asarray(inputs["vu_b1"], dtype=np.float32)
    vu_w2 = np.asarray(inputs["vu_w2"], dtype=np.float32)
    vu_b2 = np.asarray(inputs["vu_b2"], dtype=np.float32)
    er_w = np.asarray(inputs["er_w"], dtype=np.float32)
    er_b = np.asarray(inputs["er_b"], dtype=np.float32)
    ad_w = np.asarray(inputs["ad_w"], dtype=np.float32)
    ad_b = np.asarray(inputs["ad_b"], dtype=np.float32)
    out_w1 = np.asarray(inputs["out_w1"], dtype=np.float32)
    out_b1 = np.asarray(inputs["out_b1"], dtype=np.float32)
    out_w2 = np.asarray(inputs["out_w2"], dtype=np.float32)
    out_b2 = np.asarray(inputs["out_b2"], dtype=np.float32)

    w2er = (vu_w2.astype(np.float64) @ er_w.astype(np.float64)).astype(np.float32)
    w2ad = (vu_w2.astype(np.float64) @ ad_w.astype(np.float64)).astype(np.float32)
    ebf = (vu_b2.astype(np.float64) @ er_w.astype(np.float64) + er_b).astype(np.float32)
    abf = (vu_b2.astype(np.float64) @ ad_w.astype(np.float64) + ad_b).astype(np.float32)

    bf = ml_dtypes.bfloat16
    shared = {
        "emb": emb,
        "kT": np.ascontiguousarray(key_matrix.T),
        "w1r": np.ascontiguousarray(vu_w1[:D]).astype(bf),
        "w1q": np.ascontiguousarray(vu_w1[D:]).astype(bf),
        "w2er": w2er.astype(bf), "w2ad": w2ad.astype(bf),
        "b1": vu_b1.reshape(D, 1), "eb": ebf.reshape(D, 1), "ab": abf.reshape(D, 1),
        "ow1r": np.ascontiguousarray(out_w1[:D]),
        "ow1q": np.ascontiguousarray(out_w1[D:]),
        "ob1": out_b1.reshape(D, 1),
        "ow2": np.ascontiguousarray(out_w2.reshape(D, 1)),
        "ob2": out_b2.reshape(1, 1),
    }
    in_maps = []
    for c in range(NCORES):
        qidx = np.zeros((128, NQTILES), np.int32)
        for j in range(NQTILES):
            for p in range(128):
                n = j * 128 + p
                if n < S * BC:
                    s, bl = divmod(n, BC)
                    qidx[p, j] = q[c * BC + bl, s]
        m = dict(shared)
        m["qidx"] = qidx
        in_maps.append(m)
    return in_maps


def _install_ntff_shim():
    # Optional: enables NTFF hardware profiling under axon when tracing is
    # requested. Harmless no-op if the pieces are missing.
    import types, sys
    if "antenv.axon_hooks" in sys.modules:
        return
    try:
        import antenv
        from trn_agent_boot.trn_boot import _ntff_profile_via_ctypes
        hook = _ntff_profile_via_ctypes("/opt/axon/libaxon_pjrt.so")
        mod = types.ModuleType("antenv.axon_hooks")
        state = {"hook": hook}
        mod.get_axon_ntff_profile_hook = lambda: state["hook"]
        mod.set_axon_ntff_profile_hook = lambda h: state.update(hook=h)
        sys.modules["antenv.axon_hooks"] = mod
        antenv.axon_hooks = mod
    except Exception:
        pass


def kernel(**inputs) -> np.ndarray:
    if bool(int(os.environ.get("DKVMN_TRACE", "0"))):
        _install_ntff_shim()
    nc = _build_program()
    in_maps = _host_inputs(inputs)
    res = bass_utils.run_bass_kernel_spmd(
        nc, in_maps, core_ids=list(range(NCORES)),
        trace=bool(int(os.environ.get("DKVMN_TRACE", "0"))),
    )
    _CACHE["last_results"] = res
    pred = np.concatenate([res.results[c]["pred"].reshape(BC) for c in range(NCORES)])
    return pred.astype(np.float32)


# revision 17
# speedup vs baseline: 1.4369x; 1.1467x over previous
"""DKVMN forward kernel for 8 Trainium2 NeuronCores.

Data-parallel over batch: B=128 -> 16 per core. Per-core state
v[d=128 partitions, (b,m)=16*50=800 free] f32 in SBUF.

Per scan step t:
  w_exp = ones^T @ attn_row_t        (PE, f32r exact broadcast, PSUM)
  z     = v * w_exp                  (DVE)
  readT = reduce_m(z)                (DVE)          [128,16]
  hT    = tanh(W1r^T readT + W1q^T qeT + b1)   (PE + ACT)
  eT    = sigmoid(W2er^T hT + eb)    (PE + ACT)    W2er = vu_w2 @ er_w (host)
  aT    = tanh(W2ad^T hT + ab)       (PE + ACT)
  t1    = z * bcast(eT)              (DVE)
  t2    = w_exp * bcast(aT)          (DVE, -> SBUF)
  g     = t1 - v                     (GPSIMD)
  v     = t2 - g                     (GPSIMD)      = v - z*e + w*a
"""

import os
import numpy as np
import ml_dtypes
from contextlib import ExitStack

import concourse.bass as bass
import concourse.bacc as bacc
import concourse.mybir as mybir
import concourse.tile as tile
import concourse.bass_utils as bass_utils
from concourse.masks import make_identity

B, S, M, D, NQ = 128, 100, 50, 128, 10000
NCORES = 8
BC = B // NCORES          # 16 batch rows per core
BM = BC * M               # 800
NQTILES = (S * BC + 127) // 128   # 13 gather tiles
QCOLS = NQTILES * 128     # 1664

F32 = mybir.dt.float32
F32R = mybir.dt.float32r
BF16 = mybir.dt.bfloat16
I32 = mybir.dt.int32
AF = mybir.ActivationFunctionType
OP = mybir.AluOpType
AX = mybir.AxisListType

_CACHE = {}


def _build_program():
    if "nc" in _CACHE:
        return _CACHE["nc"]

    nc = bacc.Bacc("TRN2", target_bir_lowering=False, debug=False,
                   enable_asserts=False, num_devices=NCORES)

    dram_in = {}
    for name, shape, dt in [
        ("emb", [NQ, D], F32),
        ("qidx", [128, NQTILES], I32),
        ("kT", [D, M], F32),
        ("w1r", [D, D], BF16), ("w1q", [D, D], BF16),
        ("w2er", [D, D], BF16), ("w2ad", [D, D], BF16),
        ("b1", [D, 1], F32), ("eb", [D, 1], F32), ("ab", [D, 1], F32),
        ("ow1r", [D, D], F32), ("ow1q", [D, D], F32),
        ("ob1", [D, 1], F32), ("ow2", [D, 1], F32), ("ob2", [1, 1], F32),
    ]:
        dram_in[name] = nc.dram_tensor(name, shape, dt, kind="ExternalInput").ap()
    pred_out = nc.dram_tensor("pred", [1, BC], F32, kind="ExternalOutput").ap()

    with tile.TileContext(nc) as tc, ExitStack() as ctx:
        persist = ctx.enter_context(tc.tile_pool(name="persist", bufs=1))

        # ---- persistent SBUF tiles ----
        kT = persist.tile([D, M], F32, tag="kT")
        w1r = persist.tile([D, D], BF16, tag="w1r")
        w1q = persist.tile([D, D], BF16, tag="w1q")
        w2er = persist.tile([D, D], BF16, tag="w2er")
        w2ad = persist.tile([D, D], BF16, tag="w2ad")
        b1 = persist.tile([D, 1], F32, tag="b1")
        eb = persist.tile([D, 1], F32, tag="eb")
        ab = persist.tile([D, 1], F32, tag="ab")
        ow1r = persist.tile([D, D], F32, tag="ow1r")
        ow1q = persist.tile([D, D], F32, tag="ow1q")
        ob1 = persist.tile([D, 1], F32, tag="ob1")
        ow2 = persist.tile([D, 1], F32, tag="ow2")
        ob2 = persist.tile([1, 1], F32, tag="ob2")
        idx = persist.tile([128, NQTILES], I32, tag="idx")
        ident = persist.tile([128, 128], F32, tag="ident")
        identr = persist.tile([128, 128], F32R, tag="identr")
        attnr = persist.tile([S, BM], F32R, tag="attnr")
        qT = persist.tile([D, QCOLS], F32, tag="qT")
        qTb = persist.tile([D, QCOLS], BF16, tag="qTb")
        attn = persist.tile([S, BM], F32, tag="attn")
        vpp = [[persist.tile([D, 400], F32, name=f"v{g}p{p}", tag=f"v{g}p{p}")
                for p in (0, 1)] for g in (0, 1)]

        for nm, t in [("kT", kT), ("w1r", w1r), ("w1q", w1q), ("w2er", w2er),
                      ("w2ad", w2ad), ("b1", b1), ("eb", eb), ("ab", ab),
                      ("ow1r", ow1r), ("ow1q", ow1q), ("ob1", ob1),
                      ("ow2", ow2), ("ob2", ob2), ("qidx", idx)]:
            nc.sync.dma_start(t[:], dram_in[nm][:])
        make_identity(nc, ident[:])
        nc.vector.tensor_copy(identr[:], ident[:])
        nc.vector.memset(vpp[0][0][:], 0.0)
        nc.vector.memset(vpp[1][0][:], 0.0)

        # ---- phase 1: gather q_emb rows and transpose into qT ----
        with tc.tile_pool(name="gather", bufs=3) as gpool, \
             tc.tile_pool(name="tpsum", bufs=4, space="PSUM") as tpsum:
            for j in range(NQTILES):
                qg = gpool.tile([128, D], F32, tag="qg")
                nc.gpsimd.indirect_dma_start(
                    out=qg[:], out_offset=None,
                    in_=dram_in["emb"][:],
                    in_offset=bass.IndirectOffsetOnAxis(ap=idx[:, j:j + 1], axis=0),
                )
                tp = tpsum.tile([128, 128], F32, tag="tp")
                nc.tensor.transpose(tp[:], qg[:], ident[:])
                if j % 2 == 0:
                    nc.vector.tensor_copy(qT[:, j * 128:(j + 1) * 128], tp[:])
                else:
                    nc.scalar.copy(qT[:, j * 128:(j + 1) * 128], tp[:])

        nc.scalar.copy(qTb[:], qT[:])

        # ---- phase 2: scores + softmax -> attn[s, (b,m)] ----
        with tc.tile_pool(name="spsum", bufs=4, space="PSUM") as spsum:
            for b in range(BC):
                sc = spsum.tile([S, M], F32, tag="sc")
                qTsl = qT[:, b:S * BC:BC]         # [128, 100] strided (s,b) layout
                nc.tensor.matmul(sc[:], qTsl, kT[:], start=True, stop=True)
                if b % 2 == 0:
                    nc.vector.tensor_copy(attn[:, b * M:(b + 1) * M], sc[:])
                else:
                    nc.scalar.copy(attn[:, b * M:(b + 1) * M], sc[:])

        with tc.tile_pool(name="smx", bufs=1) as smx:
            a3 = attn[:].rearrange("p (b m) -> p b m", b=BC)
            mx = smx.tile([S, BC], F32, tag="mx")
            nc.vector.tensor_reduce(mx[:], a3, axis=AX.X, op=OP.max)
            mxb = mx[:, :, None].broadcast_to([S, BC, M])
            nc.vector.tensor_tensor(a3, a3, mxb, op=OP.subtract)
            nc.scalar.activation(attn[:], attn[:], AF.Exp)
            sm = smx.tile([S, BC], F32, tag="sm")
            nc.vector.tensor_reduce(sm[:], a3, axis=AX.X, op=OP.add)
            rec = smx.tile([S, BC], F32, tag="rec")
            nc.vector.reciprocal(rec[:], sm[:])
            recb = rec[:, :, None].broadcast_to([S, BC, M])
            nc.vector.tensor_tensor(a3, a3, recb, op=OP.mult)
            nc.vector.tensor_copy(attnr[:], attn[:])

        # ---- phase 3: the scan (two independent batch groups, interleaved) ----
        # Group g owns batch rows [8g, 8g+8) -> v columns [400g, 400g+400).
        GW = 400          # group width in v-columns
        GB2 = BC // 2     # 8 batch rows per group
        with tc.tile_pool(name="wide", bufs=3) as wide, \
             tc.tile_pool(name="small", bufs=4) as small, \
             tc.tile_pool(name="wexp", bufs=4, space="PSUM") as wexpp, \
             tc.tile_pool(name="mlp", bufs=2, space="PSUM") as mlpp, \
             tc.tile_pool(name="mlp2", bufs=1, space="PSUM") as mlpp2:

            def g_read(t, g, vcur):
                """wexp select-bcast + z + readT for group g at step t."""
                c0 = g * GW
                sel = identr[0:S, t:t + 1].broadcast_to([S, D])
                w = wexpp.tile([D, GW], F32, tag="w")
                nc.tensor.matmul(w[:], sel, attnr[:, c0:c0 + GW],
                                 start=True, stop=True)
                z = wide.tile([D, GW], F32, tag=f"z{g}")
                nc.vector.tensor_tensor(z[:], vcur[:], w[:], op=OP.mult)
                readT = small.tile([D, GB2], F32, tag=f"r{g}")
                z3 = z[:].rearrange("p (b m) -> p b m", b=GB2)
                nc.vector.tensor_reduce(readT[:], z3, axis=AX.X, op=OP.add)
                rbf = small.tile([D, GB2], BF16, tag=f"rb{g}")
                nc.scalar.copy(rbf[:], readT[:])
                return z, w, rbf

            def g_gates(t, g, readT):
                """MLP + gates for group g; returns eT, aT ([D, 8])."""
                qeT = qTb[:, t * BC + g * GB2: t * BC + (g + 1) * GB2]
                hps = mlpp.tile([D, GB2], F32, tag="hps")
                nc.tensor.matmul(hps[:], w1r[:], readT[:], start=True, stop=False)
                nc.tensor.matmul(hps[:], w1q[:], qeT, start=False, stop=True)
                hT = small.tile([D, GB2], BF16, tag=f"h{g}")
                nc.scalar.activation(hT[:], hps[:], AF.Tanh, bias=b1[:])
                eps = mlpp2.tile([D, GB2], F32, tag="eps")
                nc.tensor.matmul(eps[:], w2er[:], hT[:], start=True, stop=True)
                eT = small.tile([D, GB2], F32, tag=f"e{g}")
                nc.scalar.activation(eT[:], eps[:], AF.Sigmoid, bias=eb[:])
                aps = mlpp2.tile([D, GB2], F32, tag="aps")
                nc.tensor.matmul(aps[:], w2ad[:], hT[:], start=True, stop=True)
                aT = small.tile([D, GB2], F32, tag=f"a{g}")
                nc.scalar.activation(aT[:], aps[:], AF.Tanh, bias=ab[:])
                return eT, aT

            def g_update(g, z, w, eT, aT, vcur, vnext):
                """v[g] <- v[g] - z*E + w*A  via d = t1 - t2; v -= d."""
                c0 = g * GW
                z3 = z[:].rearrange("p (b m) -> p b m", b=GB2)
                ebc = eT[:, :, None].broadcast_to([D, GB2, M])
                t1 = wide.tile([D, GW], F32, tag=f"t1{g}")
                nc.vector.tensor_tensor(t1[:].rearrange("p (b m) -> p b m", b=GB2),
                                        z3, ebc, op=OP.mult)
                abc = aT[:, :, None].broadcast_to([D, GB2, M])
                t2 = wide.tile([D, GW], F32, tag=f"t2{g}")
                nc.vector.tensor_tensor(
                    t2[:].rearrange("p (b m) -> p b m", b=GB2),
                    w[:].rearrange("p (b m) -> p b m", b=GB2), abc, op=OP.mult)
                d = wide.tile([D, GW], F32, tag=f"d{g}")
                nc.gpsimd.tensor_tensor(d[:], t1[:], t2[:], op=OP.subtract)
                nc.gpsimd.tensor_tensor(vnext[:], vcur[:], d[:], op=OP.subtract)

            state = [None, None]
            z, w, readT = g_read(0, 0, vpp[0][0])
            eT, aT = g_gates(0, 0, readT)
            state[0] = (z, w, eT, aT)
            for t in range(S):
                pc, pn = t % 2, (t + 1) % 2
                g_update(0, *state[0], vpp[0][pc], vpp[0][pn])
                z, w, readT = g_read(t, 1, vpp[1][pc])
                eT, aT = g_gates(t, 1, readT)
                state[1] = (z, w, eT, aT)
                g_update(1, *state[1], vpp[1][pc], vpp[1][pn])
                if t + 1 < S:
                    z, w, readT = g_read(t + 1, 0, vpp[0][pn])
                    eT, aT = g_gates(t + 1, 0, readT)
                    state[0] = (z, w, eT, aT)

            # ---- final prediction ----
            readF = small.tile([D, BC], F32, tag="readF")
            for g in (0, 1):
                c0 = g * GW
                sel = identr[0:S, S - 1:S].broadcast_to([S, D])
                w = wexpp.tile([D, GW], F32, tag="w")
                nc.tensor.matmul(w[:], sel, attnr[:, c0:c0 + GW],
                                 start=True, stop=True)
                zf = wide.tile([D, GW], F32, tag=f"z{g}")
                nc.vector.tensor_tensor(zf[:], vpp[g][S % 2][:], w[:], op=OP.mult)
                z3 = zf[:].rearrange("p (b m) -> p b m", b=GB2)
                nc.vector.tensor_reduce(readF[:, g * GB2:(g + 1) * GB2], z3,
                                        axis=AX.X, op=OP.add)
            qeT = qT[:, (S - 1) * BC:S * BC]
            h2ps = mlpp.tile([D, BC], F32, tag="hps")
            nc.tensor.matmul(h2ps[:], ow1r[:], readF[:], start=True, stop=False)
            nc.tensor.matmul(h2ps[:], ow1q[:], qeT, start=False, stop=True)
            h2 = small.tile([D, BC], F32, tag="h0")
            nc.scalar.activation(h2[:], h2ps[:], AF.Relu, bias=ob1[:])
            pps = mlpp2.tile([1, BC], F32, tag="eps")
            nc.tensor.matmul(pps[:], ow2[:], h2[:], start=True, stop=True)
            ps = small.tile([1, BC], F32, tag="pred")
            nc.scalar.activation(ps[:], pps[:], AF.Sigmoid, bias=ob2[:])
            nc.sync.dma_start(pred_out[:], ps[:])

    nc.compile()
    _CACHE["nc"] = nc
    return nc


def _host_inputs(inputs):
    """Per-core input maps from the full problem inputs."""
    q = np.asarray(inputs["question_seq"]).astype(np.int64)
    emb = np.ascontiguousarray(np.asarray(inputs["emb"], dtype=np.float32))
    key_matrix = np.asarray(inputs["key_matrix"], dtype=np.float32)
    vu_w1 = np.asarray(inputs["vu_w1"], dtype=np.float32)
    vu_b1 = np.asarray(inputs["vu_b1"], dtype=np.float32)
    vu_w2 = np.asarray(inputs["vu_w2"], dtype=np.float32)
    vu_b2 = np.asarray(inputs["vu_b2"], dtype=np.float32)
    er_w = np.asarray(inputs["er_w"], dtype=np.float32)
    er_b = np.asarray(inputs["er_b"], dtype=np.float32)
    ad_w = np.asarray(inputs["ad_w"], dtype=np.float32)
    ad_b = np.asarray(inputs["ad_b"], dtype=np.float32)
    out_w1 = np.asarray(inputs["out_w1"], dtype=np.float32)
    out_b1 = np.asarray(inputs["out_b1"], dtype=np.float32)
    out_w2 = np.asarray(inputs["out_w2"], dtype=np.float32)
    out_b2 = np.asarray(inputs["out_b2"], dtype=np.float32)

    w2er = (vu_w2.astype(np.float64) @ er_w.astype(np.float64)).astype(np.float32)
    w2ad = (vu_w2.astype(np.float64) @ ad_w.astype(np.float64)).astype(np.float32)
    ebf = (vu_b2.astype(np.float64) @ er_w.astype(np.float64) + er_b).astype(np.float32)
    abf = (vu_b2.astype(np.float64) @ ad_w.astype(np.float64) + ad_b).astype(np.float32)

    bf = ml_dtypes.bfloat16
    shared = {
        "emb": emb,
        "kT": np.ascontiguousarray(key_matrix.T),
        "w1r": np.ascontiguousarray(vu_w1[:D]).astype(bf),
        "w1q": np.ascontiguousarray(vu_w1[D:]).astype(bf),
        "w2er": w2er.astype(bf), "w2ad": w2ad.astype(bf),
        "b1": vu_b1.reshape(D, 1), "eb": ebf.reshape(D, 1), "ab": abf.reshape(D, 1),
        "ow1r": np.ascontiguousarray(out_w1[:D]),
        "ow1q": np.ascontiguousarray(out_w1[D:]),
        "ob1": out_b1.reshape(D, 1),
        "ow2": np.ascontiguousarray(out_w2.reshape(D, 1)),
        "ob2": out_b2.reshape(1, 1),
    }
    in_maps = []
    for c in range(NCORES):
        qidx = np.zeros((128, NQTILES), np.int32)
        for j in range(NQTILES):
            for p in range(128):
                n = j * 128 + p
                if n < S * BC:
                    s, bl = divmod(n, BC)
                    qidx[p, j] = q[c * BC + bl, s]
        m = dict(shared)
        m["qidx"] = qidx
        in_maps.append(m)
    return in_maps


def _install_ntff_shim():
    # Optional: enables NTFF hardware profiling under axon when tracing is
    # requested. Harmless no-op if the pieces are missing.
    import types, sys
    if "antenv.axon_hooks" in sys.modules:
        return
    try:
        import antenv
        from trn_agent_boot.trn_boot import _ntff_profile_via_ctypes
        hook = _ntff_profile_via_ctypes("/opt/axon/libaxon_pjrt.so")
        mod = types.ModuleType("antenv.axon_hooks")
        state = {"hook": hook}
        mod.get_axon_ntff_profile_hook = lambda: state["hook"]
        mod.set_axon_ntff_profile_hook = lambda h: state.update(hook=h)
        sys.modules["antenv.axon_hooks"] = mod
        antenv.axon_hooks = mod
    except Exception:
        pass


def kernel(**inputs) -> np.ndarray:
    if bool(int(os.environ.get("DKVMN_TRACE", "0"))):
        _install_ntff_shim()
    nc = _build_program()
    in_maps = _host_inputs(inputs)
    res = bass_utils.run_bass_kernel_spmd(
        nc, in_maps, core_ids=list(range(NCORES)),
        trace=bool(int(os.environ.get("DKVMN_TRACE", "0"))),
    )
    _CACHE["last_results"] = res
    pred = np.concatenate([res.results[c]["pred"].reshape(BC) for c in range(NCORES)])
    return pred.astype(np.float32)
